# revision 14
# baseline (speedup 1.0000x reference)
"""Trainium2 Bass kernel for nn_ContextAttention (vq_codebook).

Strategy: data-parallel over the batch (B=64) across 8 NeuronCores, 8 samples
per core.  The tiny routing stage (l2-normalize CLS, 4-way argmax, codebook
gather) runs on the host in fp32; everything else (cross-attention pooling,
k/v/q projections, 12-head MHSA, output projection) runs on-device.

Device-side layout choices (per sample):
  - host pre-transposes x -> xT [D, N] so every D-contraction matmul has D on
    partitions without any device transpose of x.
  - cross-attn softmax is computed WITHOUT max subtraction (scores are bounded
    ~|3.5|) in transposed layout scoresT [P, K]; the softmax denominator is
    obtained for free as an extra ones-column in the ctx matmul and folded
    into the PSUM eviction as a per-partition scale.
  - MHSA is computed as attT [m=133, N] per head (m on partitions), exp with
    the 1/8 scale folded into the ACT activation, denominator obtained by
    augmenting V with a ones column, and the per-(head, n) normalization is
    applied during a PE-transpose round-trip (yT -> y_nat(scale) -> yT).
All matmul operands are bf16 (fp32 PSUM accumulation); vector math fp32.
"""
import sys

sys.path.insert(0, "/opt/trn_rl_repo")

import numpy as np
import ml_dtypes

import concourse.bass as bass
import concourse.mybir as mybir
import concourse.tile as tile
from concourse import bacc
from concourse.bass_utils import run_bass_kernel_spmd

BF16 = ml_dtypes.bfloat16

B, N, D = 64, 1029, 768
H, DH = 12, 64
K, R, M = 128, 5, 4
P = N - R            # 1024 patches
NCORES = 8
SPC = B // NCORES    # samples per core
DC = D // 128        # 6 chunks of the model dim
PC = P // 128        # 8 chunks of the patch dim
NF = 343             # N = 3 * 343 free-dim split for big matmuls
NCH = [128] * 8 + [5]  # N partition-chunks: 8*128 + 5

_bf = mybir.dt.bfloat16
_f32 = mybir.dt.float32

# set by test harness: trace the SPMD run and record exec time
TRACE = False
LAST_RESULT = None


def build_nc(samples=SPC, num_cores=NCORES):
    # Bacc (not plain Bass): its compile() runs move_matmul_waits_to_ldweights
    # + generate_event_semaphores, required by TRN2's 1-wait-per-instruction
    # limit (walrus rejects 2-wait matmuls otherwise).
    nc = bacc.Bacc("TRN2", target_bir_lowering=False, debug=False,
                   num_devices=num_cores)

    xT = nc.declare_dram_parameter("xT", [samples, D, N], _bf, isOutput=False)
    xpa = nc.declare_dram_parameter("xpa", [samples, P, D + 1], _bf, isOutput=False)
    qcT = nc.declare_dram_parameter("qcT", [samples, D, K], _bf, isOutput=False)
    wq = nc.declare_dram_parameter("wq", [D, D], _bf, isOutput=False)
    wctx = nc.declare_dram_parameter("wctx", [D, 2 * D], _bf, isOutput=False)
    wout = nc.declare_dram_parameter("wout", [D, D], _bf, isOutput=False)
    bk = nc.declare_dram_parameter("bk", [128, DC], _f32, isOutput=False)
    bvb = nc.declare_dram_parameter("bvb", [128, D], _f32, isOutput=False)
    bob = nc.declare_dram_parameter("bob", [128, D], _f32, isOutput=False)
    identb = nc.declare_dram_parameter("identb", [128, 128], _bf, isOutput=False)
    identf = nc.declare_dram_parameter("identf", [128, 128], _f32, isOutput=False)
    out = nc.declare_dram_parameter("out", [samples, N, D], _f32, isOutput=True)

    with tile.TileContext(nc) as tc:
        with (
            tc.tile_pool(name="wpool", bufs=1) as wpool,
            tc.tile_pool(name="inpool", bufs=2) as inpool,
            tc.tile_pool(name="mid", bufs=1) as mid,
            tc.tile_pool(name="qpool", bufs=2) as qpool,
            tc.tile_pool(name="xpap", bufs=1) as xpap,
            tc.tile_pool(name="small", bufs=2) as small,
            tc.tile_pool(name="attp", bufs=2) as attp,
            tc.tile_pool(name="outp", bufs=2) as outp,
            tc.tile_pool(name="pp", bufs=3, space="PSUM") as pp,
            tc.tile_pool(name="pa", bufs=3, space="PSUM") as pa,
            tc.tile_pool(name="ppt", bufs=2, space="PSUM") as ppt,
        ):
            # ---- persistent tiles --------------------------------------
            wq_sb = wpool.tile([128, DC, D], _bf)
            nc.sync.dma_start(wq_sb[:], wq.rearrange("(dc p) q -> p dc q", p=128))
            wctx_sb = wpool.tile([128, DC, 2 * D], _bf)
            nc.sync.dma_start(wctx_sb[:], wctx.rearrange("(dc p) q -> p dc q", p=128))
            wout_sb = wpool.tile([128, DC, D], _bf)
            nc.sync.dma_start(wout_sb[:], wout.rearrange("(dc p) q -> p dc q", p=128))
            bk_sb = wpool.tile([128, DC], _f32)
            nc.sync.dma_start(bk_sb[:], bk[:])
            bvb_sb = wpool.tile([128, D], _f32)
            nc.sync.dma_start(bvb_sb[:], bvb[:])
            bob_sb = wpool.tile([128, D], _f32)
            nc.sync.dma_start(bob_sb[:], bob[:])
            idb_sb = wpool.tile([128, 128], _bf)
            nc.sync.dma_start(idb_sb[:], identb[:])
            idf_sb = wpool.tile([128, 128], _f32)
            nc.sync.dma_start(idf_sb[:], identf[:])

            for b in range(samples):
                # ---- per-sample inputs ---------------------------------
                xT_sb = inpool.tile([128, DC, N], _bf, tag="xT")
                nc.sync.dma_start(
                    xT_sb[:], xT[b].rearrange("(dc p) n -> p dc n", p=128))
                xpa_sb = xpap.tile([128, PC, D + 1], _bf, tag="xpa")
                nc.sync.dma_start(
                    xpa_sb[:], xpa[b].rearrange("(pc p) e -> p pc e", p=128))
                qcT_sb = inpool.tile([128, DC, K], _bf, tag="qcT")
                nc.sync.dma_start(
                    qcT_sb[:], qcT[b].rearrange("(dc p) k -> p dc k", p=128))

                # ---- S2/S3: scoresT + E (cross-attn, transposed) -------
                E_sb = mid.tile([128, PC, K], _bf, tag="E")
                for pc in range(PC):
                    ps_sc = pp.tile([128, 512], _f32, tag="ps", name="ps")[:, :K]
                    for dc in range(DC):
                        nc.tensor.matmul(
                            ps_sc,
                            lhsT=xT_sb[:, dc, R + 128 * pc:R + 128 * (pc + 1)],
                            rhs=qcT_sb[:, dc, :],
                            start=(dc == 0), stop=(dc == DC - 1))
                    nc.scalar.activation(
                        E_sb[:, pc, :], ps_sc, mybir.ActivationFunctionType.Exp)

                # ---- S4-S6: ctx_p = (E^T @ [xp | 1]) * (1/s) -----------
                psA = pp.tile([128, 512], _f32, tag="ps", name="ps")[:, :384]
                psB = pp.tile([128, 512], _f32, tag="ps", name="ps")[:, :385]
                for pc in range(PC):
                    nc.tensor.matmul(psA, lhsT=E_sb[:, pc, :],
                                     rhs=xpa_sb[:, pc, 0:384],
                                     start=(pc == 0), stop=(pc == PC - 1))
                for pc in range(PC):
                    nc.tensor.matmul(psB, lhsT=E_sb[:, pc, :],
                                     rhs=xpa_sb[:, pc, 384:769],
                                     start=(pc == 0), stop=(pc == PC - 1))
                r_sb = small.tile([128, 1], _f32, tag="r")
                nc.vector.reciprocal(r_sb[:], psB[:, 384:385])
                ctxp_sb = mid.tile([128, D], _bf, tag="ctxp")
                nc.scalar.activation(ctxp_sb[:, 0:384], psA,
                                     mybir.ActivationFunctionType.Copy,
                                     scale=r_sb[:])
                nc.scalar.activation(ctxp_sb[:, 384:768], psB[:, 0:384],
                                     mybir.ActivationFunctionType.Copy,
                                     scale=r_sb[:])

                # ---- S7: ctxT = [xregT | ctx_p^T] ----------------------
                ctxT_sb = mid.tile([128, DC, R + K], _bf, tag="ctxT")
                for dc in range(DC):
                    nc.vector.tensor_copy(ctxT_sb[:, dc, 0:R],
                                          xT_sb[:, dc, 0:R])
                    pst = ppt.tile([128, 128], _bf, tag="pt", name="pst")
                    nc.tensor.transpose(
                        pst[:], ctxp_sb[:, 128 * dc:128 * (dc + 1)], idb_sb[:])
                    nc.vector.tensor_copy(ctxT_sb[:, dc, R:R + K], pst[:])

                # ---- S8: kT = (ctx @ Wk)^T + bk ------------------------
                kT_sb = mid.tile([128, DC, R + K], _bf, tag="kT")
                for dkv in range(DC):
                    psk = pp.tile([128, 512], _f32, tag="ps", name="ps")[:, :R + K]
                    for dc in range(DC):
                        nc.tensor.matmul(
                            psk,
                            lhsT=wctx_sb[:, dc, 128 * dkv:128 * (dkv + 1)],
                            rhs=ctxT_sb[:, dc, :],
                            start=(dc == 0), stop=(dc == DC - 1))
                    nc.scalar.activation(kT_sb[:, dkv, :], psk,
                                         mybir.ActivationFunctionType.Identity,
                                         bias=bk_sb[:, dkv:dkv + 1])

                # ---- S9: v natural + ones col (vaug) -------------------
                vaug0 = mid.tile([128, H, DH + 1], _bf, tag="vaug0")
                vaug1 = mid.tile([128, H, DH + 1], _bf, tag="vaug1")
                nc.vector.memset(vaug0[:, :, DH:DH + 1], 1.0)
                nc.vector.memset(vaug1[0:R, :, DH:DH + 1], 1.0)
                for mc, (vt, rows, msl) in enumerate(
                        [(vaug0, 128, slice(0, 128)),
                         (vaug1, R, slice(128, 133))]):
                    for hf in range(2):
                        psv = pp.tile([128, 512], _f32, tag="ps", name="ps")[:rows, :384]
                        for dc in range(DC):
                            nc.tensor.matmul(
                                psv,
                                lhsT=ctxT_sb[:, dc, msl],
                                rhs=wctx_sb[:, dc, D + 384 * hf:D + 384 * (hf + 1)],
                                start=(dc == 0), stop=(dc == DC - 1))
                        nc.vector.tensor_add(
                            vt[0:rows, 6 * hf:6 * (hf + 1), 0:DH],
                            psv.rearrange("p (h e) -> p h e", e=DH),
                            bvb_sb[0:rows, 384 * hf:384 * (hf + 1)]
                            .rearrange("p (h e) -> p h e", e=DH))

                # ---- S10: qT = (x @ Wq)^T ------------------------------
                # double-buffered so the next sample's q-projection can
                # overlap this sample's MHSA/normalization (keeps PE warm)
                qT_sb = qpool.tile([128, DC, N], _bf, tag="qT")
                for dq in range(DC):
                    for nf in range(3):
                        psq = pp.tile([128, 512], _f32, tag="ps", name="ps")[:, :NF]
                        for dc in range(DC):
                            nc.tensor.matmul(
                                psq,
                                lhsT=wq_sb[:, dc, 128 * dq:128 * (dq + 1)],
                                rhs=xT_sb[:, dc, NF * nf:NF * (nf + 1)],
                                start=(dc == 0), stop=(dc == DC - 1))
                        nc.scalar.activation(
                            qT_sb[:, dq, NF * nf:NF * (nf + 1)], psq,
                            mybir.ActivationFunctionType.Copy)

                # ---- S11: MHSA, head pairs (row-groups 0/64 overlap) ---
                # head h's softmax denominator row lives at partition
                # 32*(h%4), free slot h//4 (32-aligned bases only)
                yTu_sb = qpool.tile([128, DC, N], _bf, tag="yTu")
                S_sb = qpool.tile([128, 3, N], _f32, tag="S")
                nc.gpsimd.memset(S_sb[:], 1.0)
                for hp in range(H // 2):
                    heads = (2 * hp, 2 * hp + 1)
                    Ea0 = {h: attp.tile([128, N], _bf, tag=f"Ea0_{h % 2}",
                                        name="Ea0") for h in heads}
                    Ea1 = {h: attp.tile([128, N], _bf, tag=f"Ea1_{h % 2}",
                                        name="Ea1") for h in heads}
                    for nf in range(3):
                        nsl = slice(NF * nf, NF * (nf + 1))
                        ps0 = {}
                        for h in heads:
                            off = DH * (h % 2)
                            ps0[h] = pa.tile([128, 512], _f32, tag="pa",
                                             name="ps0")[:, :NF]
                            nc.tensor.matmul(
                                ps0[h],
                                lhsT=kT_sb[off:off + DH, hp, 0:128],
                                rhs=qT_sb[off:off + DH, hp, nsl],
                                start=True, stop=True)
                        for h in heads:
                            nc.scalar.activation(
                                Ea0[h][:, nsl], ps0[h],
                                mybir.ActivationFunctionType.Exp, scale=0.125)
                        ps1 = {}
                        for h in heads:
                            off = DH * (h % 2)
                            ps1[h] = pa.tile([128, 512], _f32, tag="pa",
                                             name="ps1")[:R, :NF]
                            nc.tensor.matmul(
                                ps1[h],
                                lhsT=kT_sb[off:off + DH, hp, 128:133],
                                rhs=qT_sb[off:off + DH, hp, nsl],
                                start=True, stop=True)
                        for h in heads:
                            nc.scalar.activation(
                                Ea1[h][0:R, nsl], ps1[h],
                                mybir.ActivationFunctionType.Exp, scale=0.125)
                    for nf in range(3):
                        nsl = slice(NF * nf, NF * (nf + 1))
                        for h in heads:
                            off = DH * (h % 2)
                            psy = pa.tile([128, 512], _f32, tag="pa",
                                          name="psy")[:DH + 1, :NF]
                            nc.tensor.matmul(psy, lhsT=vaug0[:, h, :],
                                             rhs=Ea0[h][:, nsl],
                                             start=True, stop=False)
                            nc.tensor.matmul(psy, lhsT=vaug1[0:R, h, :],
                                             rhs=Ea1[h][0:R, nsl],
                                             start=False, stop=True)
                            nc.vector.tensor_copy(
                                yTu_sb[off:off + DH, hp, nsl], psy[0:DH, :])
                            sp = 32 * (h % 4)
                            nc.vector.tensor_copy(
                                S_sb[sp:sp + 1, h // 4, nsl],
                                psy[DH:DH + 1, :])

                # ---- S12-S15: per N-chunk: RT, normalize via transpose
                # round-trip (in place in yTu), out-projection, DMA out.
                # ncn-major so the fat out-proj matmuls interleave with the
                # transpose/evict ping-pong and keep the PE warm.
                RT_sb = qpool.tile([128, len(NCH), H], _f32, tag="RT")
                for ncn, pn in enumerate(NCH):
                    nsl = slice(128 * ncn, 128 * ncn + pn)
                    ST_n = small.tile([128, H], _f32, tag="STn")
                    for slot in range(3):
                        psf = ppt.tile([128, 128], _f32, tag="pt", name="psf")
                        nc.tensor.transpose(psf[0:pn, 0:97],
                                            S_sb[0:97, slot, nsl],
                                            idf_sb[0:97, 0:97])
                        nc.vector.tensor_copy(
                            ST_n[0:pn, 4 * slot:4 * slot + 4],
                            psf[0:pn, 0:97:32])
                    nc.vector.reciprocal(RT_sb[0:pn, ncn, :], ST_n[0:pn, :])

                    for dc in range(DC):
                        ps1 = ppt.tile([128, 128], _bf, tag="pt", name="ps1")
                        nc.tensor.transpose(ps1[0:pn, :],
                                            yTu_sb[:, dc, nsl], idb_sb[:])
                        ynat = small.tile([128, 128], _bf, tag="ynat")
                        for hh in range(2):
                            nc.scalar.activation(
                                ynat[0:pn, DH * hh:DH * (hh + 1)],
                                ps1[0:pn, DH * hh:DH * (hh + 1)],
                                mybir.ActivationFunctionType.Copy,
                                scale=RT_sb[0:pn, ncn, 2 * dc + hh:2 * dc + hh + 1])
                        ps2 = ppt.tile([128, 128], _bf, tag="pt", name="ps2")
                        nc.tensor.transpose(ps2[:, 0:pn], ynat[0:pn, :],
                                            idb_sb[0:pn, 0:pn])
                        nc.vector.tensor_copy(yTu_sb[:, dc, nsl], ps2[:, 0:pn])

                    o_sb = outp.tile([128, D], _f32, tag="osb")
                    for hf in range(2):
                        pso = pp.tile([128, 512], _f32, tag="ps", name="ps")[:pn, :384]
                        for dc in range(DC):
                            nc.tensor.matmul(
                                pso,
                                lhsT=yTu_sb[:, dc, nsl],
                                rhs=wout_sb[:, dc, 384 * hf:384 * (hf + 1)],
                                start=(dc == 0), stop=(dc == DC - 1))
                        nc.vector.tensor_add(
                            o_sb[0:pn, 384 * hf:384 * (hf + 1)], pso,
                            bob_sb[0:pn, 384 * hf:384 * (hf + 1)])
                    nc.sync.dma_start(out[b, nsl, :], o_sb[0:pn, :])

    nc.compile()
    return nc


def kernel(x, Q_banks, Wq, Wctx, bctx, Wout, bout, centroids):
    global LAST_RESULT
    x = np.asarray(x, np.float32)
    cls = x[:, 0, :]
    nrm = np.linalg.norm(cls, axis=-1, keepdims=True)
    cls_n = cls / np.maximum(nrm, 1e-12)
    sims = cls_n @ np.asarray(centroids, np.float32).T
    idx = np.argmax(sims, axis=-1).astype(np.int32)
    q_ctx = np.asarray(Q_banks, np.float32)[idx]          # [B, K, D]

    xT = np.ascontiguousarray(x.transpose(0, 2, 1)).astype(BF16)
    xpa = np.empty((B, P, D + 1), BF16)
    xpa[:, :, :D] = x[:, R:, :].astype(BF16)
    xpa[:, :, D] = np.asarray(1.0, BF16)
    qcT = np.ascontiguousarray(q_ctx.transpose(0, 2, 1)).astype(BF16)

    bctx = np.asarray(bctx, np.float32)
    bk = np.ascontiguousarray(bctx[:D].reshape(DC, 128).T).astype(np.float32)
    bvb = np.ascontiguousarray(
        np.broadcast_to(bctx[D:], (128, D))).astype(np.float32)
    bob = np.ascontiguousarray(
        np.broadcast_to(np.asarray(bout, np.float32), (128, D))).astype(np.float32)
    shared = {
        "wq": np.asarray(Wq, np.float32).astype(BF16),
        "wctx": np.asarray(Wctx, np.float32).astype(BF16),
        "wout": np.asarray(Wout, np.float32).astype(BF16),
        "bk": bk, "bvb": bvb, "bob": bob,
        "identb": np.eye(128, dtype=np.float32).astype(BF16),
        "identf": np.eye(128, dtype=np.float32),
    }

    nc = build_nc(SPC)
    in_maps = []
    for c in range(NCORES):
        sl = slice(SPC * c, SPC * (c + 1))
        in_maps.append({
            "xT": xT[sl], "xpa": xpa[sl], "qcT": qcT[sl], **shared})

    res = run_bass_kernel_spmd(
        nc, in_maps, list(range(NCORES)), trace=TRACE)
    LAST_RESULT = res
    out = np.concatenate(
        [np.asarray(res.results[c]["out"], np.float32) for c in range(NCORES)],
        axis=0)
    return out, cls_n, idx


# revision 19
# speedup vs baseline: 1.0669x; 1.0669x over previous
"""Trainium2 Bass kernel for nn_ContextAttention (vq_codebook).

Strategy: data-parallel over the batch (B=64) across 8 NeuronCores, 8 samples
per core.  The tiny routing stage (l2-normalize CLS, 4-way argmax, codebook
gather) runs on the host in fp32; everything else (cross-attention pooling,
k/v/q projections, 12-head MHSA, output projection) runs on-device.

Device-side layout choices (per sample):
  - host pre-transposes x -> xT [D, N] so every D-contraction matmul has D on
    partitions without any device transpose of x.
  - cross-attn softmax is computed WITHOUT max subtraction (scores are bounded
    ~|3.5|) in transposed layout scoresT [P, K]; the softmax denominator is
    obtained for free as an extra ones-column in the ctx matmul and folded
    into the PSUM eviction as a per-partition scale.
  - MHSA is computed as attT [m=133, N] per head (m on partitions), exp with
    the 1/8 scale folded into the ACT activation, denominator obtained by
    augmenting V with a ones column, and the per-(head, n) normalization is
    applied during a PE-transpose round-trip (yT -> y_nat(scale) -> yT).
All matmul operands are bf16 (fp32 PSUM accumulation); vector math fp32.
"""
import sys

sys.path.insert(0, "/opt/trn_rl_repo")

import numpy as np
import ml_dtypes

import concourse.bass as bass
import concourse.mybir as mybir
import concourse.tile as tile
from concourse import bacc
from concourse.bass_utils import run_bass_kernel_spmd

BF16 = ml_dtypes.bfloat16

B, N, D = 64, 1029, 768
H, DH = 12, 64
K, R, M = 128, 5, 4
P = N - R            # 1024 patches
NCORES = 8
SPC = B // NCORES    # samples per core
DC = D // 128        # 6 chunks of the model dim
PC = P // 128        # 8 chunks of the patch dim
NF = 343             # N = 3 * 343 free-dim split for big matmuls
NCH = [128] * 8 + [5]  # N partition-chunks: 8*128 + 5

_bf = mybir.dt.bfloat16
_f32 = mybir.dt.float32

# set by test harness: trace the SPMD run and record exec time
TRACE = False
LAST_RESULT = None


def build_nc(samples=SPC, num_cores=NCORES):
    # Bacc (not plain Bass): its compile() runs move_matmul_waits_to_ldweights
    # + generate_event_semaphores, required by TRN2's 1-wait-per-instruction
    # limit (walrus rejects 2-wait matmuls otherwise).
    nc = bacc.Bacc("TRN2", target_bir_lowering=False, debug=False,
                   num_devices=num_cores)

    xT = nc.declare_dram_parameter("xT", [samples, D, N], _bf, isOutput=False)
    xpa = nc.declare_dram_parameter("xpa", [samples, P, D + 1], _bf, isOutput=False)
    qcT = nc.declare_dram_parameter("qcT", [samples, D, K], _bf, isOutput=False)
    wq = nc.declare_dram_parameter("wq", [D, D], _bf, isOutput=False)
    wctx = nc.declare_dram_parameter("wctx", [D, 2 * D], _bf, isOutput=False)
    wout = nc.declare_dram_parameter("wout", [D, D], _bf, isOutput=False)
    bk = nc.declare_dram_parameter("bk", [128, DC], _f32, isOutput=False)
    bvb = nc.declare_dram_parameter("bvb", [128, D], _f32, isOutput=False)
    bob = nc.declare_dram_parameter("bob", [128, D], _f32, isOutput=False)
    identb = nc.declare_dram_parameter("identb", [128, 128], _bf, isOutput=False)
    identf = nc.declare_dram_parameter("identf", [128, 128], _f32, isOutput=False)
    out = nc.declare_dram_parameter("out", [samples, N, D], _f32, isOutput=True)

    with tile.TileContext(nc) as tc:
        with (
            tc.tile_pool(name="wpool", bufs=1) as wpool,
            tc.tile_pool(name="inpool", bufs=2) as inpool,
            tc.tile_pool(name="mid", bufs=1) as mid,
            tc.tile_pool(name="qpool", bufs=2) as qpool,
            tc.tile_pool(name="xpap", bufs=1) as xpap,
            tc.tile_pool(name="small", bufs=2) as small,
            tc.tile_pool(name="attp", bufs=2) as attp,
            tc.tile_pool(name="eap", bufs=1) as eap,
            tc.tile_pool(name="outp", bufs=2) as outp,
            tc.tile_pool(name="pp", bufs=2, space="PSUM") as pp,
            tc.tile_pool(name="pa", bufs=4, space="PSUM") as pa,
            tc.tile_pool(name="ppt", bufs=2, space="PSUM") as ppt,
        ):
            # ---- persistent tiles --------------------------------------
            wq_sb = wpool.tile([128, DC, D], _bf)
            nc.sync.dma_start(wq_sb[:], wq.rearrange("(dc p) q -> p dc q", p=128))
            wctx_sb = wpool.tile([128, DC, 2 * D], _bf)
            nc.sync.dma_start(wctx_sb[:], wctx.rearrange("(dc p) q -> p dc q", p=128))
            wout_sb = wpool.tile([128, DC, D], _bf)
            nc.sync.dma_start(wout_sb[:], wout.rearrange("(dc p) q -> p dc q", p=128))
            bk_sb = wpool.tile([128, DC], _f32)
            nc.sync.dma_start(bk_sb[:], bk[:])
            bvb_sb = wpool.tile([128, D], _f32)
            nc.sync.dma_start(bvb_sb[:], bvb[:])
            bob_sb = wpool.tile([128, D], _f32)
            nc.sync.dma_start(bob_sb[:], bob[:])
            idb_sb = wpool.tile([128, 128], _bf)
            nc.sync.dma_start(idb_sb[:], identb[:])
            idf_sb = wpool.tile([128, 128], _f32)
            nc.sync.dma_start(idf_sb[:], identf[:])

            for b in range(samples):
                # ---- per-sample inputs ---------------------------------
                xT_sb = inpool.tile([128, DC, N], _bf, tag="xT")
                nc.sync.dma_start(
                    xT_sb[:], xT[b].rearrange("(dc p) n -> p dc n", p=128))
                xpa_sb = xpap.tile([128, PC, D + 1], _bf, tag="xpa")
                nc.sync.dma_start(
                    xpa_sb[:], xpa[b].rearrange("(pc p) e -> p pc e", p=128))
                qcT_sb = inpool.tile([128, DC, K], _bf, tag="qcT")
                nc.sync.dma_start(
                    qcT_sb[:], qcT[b].rearrange("(dc p) k -> p dc k", p=128))

                # ---- S2/S3: scoresT + E (cross-attn, transposed) -------
                E_sb = mid.tile([128, PC, K], _bf, tag="E")
                for pc in range(PC):
                    ps_sc = pp.tile([128, 512], _f32, tag="ps", name="ps")[:, :K]
                    for dc in range(DC):
                        nc.tensor.matmul(
                            ps_sc,
                            lhsT=xT_sb[:, dc, R + 128 * pc:R + 128 * (pc + 1)],
                            rhs=qcT_sb[:, dc, :],
                            start=(dc == 0), stop=(dc == DC - 1))
                    nc.scalar.activation(
                        E_sb[:, pc, :], ps_sc, mybir.ActivationFunctionType.Exp)

                # ---- S4-S6: ctx_p = (E^T @ [xp | 1]) * (1/s) -----------
                psA = pp.tile([128, 512], _f32, tag="ps", name="ps")[:, :384]
                psB = pp.tile([128, 512], _f32, tag="ps", name="ps")[:, :385]
                for pc in range(PC):
                    nc.tensor.matmul(psA, lhsT=E_sb[:, pc, :],
                                     rhs=xpa_sb[:, pc, 0:384],
                                     start=(pc == 0), stop=(pc == PC - 1))
                for pc in range(PC):
                    nc.tensor.matmul(psB, lhsT=E_sb[:, pc, :],
                                     rhs=xpa_sb[:, pc, 384:769],
                                     start=(pc == 0), stop=(pc == PC - 1))
                r_sb = small.tile([128, 1], _f32, tag="r")
                nc.vector.reciprocal(r_sb[:], psB[:, 384:385])
                ctxp_sb = mid.tile([128, D], _bf, tag="ctxp")
                nc.scalar.activation(ctxp_sb[:, 0:384], psA,
                                     mybir.ActivationFunctionType.Copy,
                                     scale=r_sb[:])
                nc.scalar.activation(ctxp_sb[:, 384:768], psB[:, 0:384],
                                     mybir.ActivationFunctionType.Copy,
                                     scale=r_sb[:])

                # ---- S7: ctxT = [xregT | ctx_p^T] ----------------------
                ctxT_sb = mid.tile([128, DC, R + K], _bf, tag="ctxT")
                for dc in range(DC):
                    nc.vector.tensor_copy(ctxT_sb[:, dc, 0:R],
                                          xT_sb[:, dc, 0:R])
                    pst = ppt.tile([128, 128], _bf, tag="pt", name="pst")
                    nc.tensor.transpose(
                        pst[:], ctxp_sb[:, 128 * dc:128 * (dc + 1)], idb_sb[:])
                    nc.vector.tensor_copy(ctxT_sb[:, dc, R:R + K], pst[:])

                # ---- S8: kT = (ctx @ Wk)^T + bk ------------------------
                kT_sb = mid.tile([128, DC, R + K], _bf, tag="kT")
                for dkv in range(DC):
                    psk = pp.tile([128, 512], _f32, tag="ps", name="ps")[:, :R + K]
                    for dc in range(DC):
                        nc.tensor.matmul(
                            psk,
                            lhsT=wctx_sb[:, dc, 128 * dkv:128 * (dkv + 1)],
                            rhs=ctxT_sb[:, dc, :],
                            start=(dc == 0), stop=(dc == DC - 1))
                    nc.scalar.activation(kT_sb[:, dkv, :], psk,
                                         mybir.ActivationFunctionType.Identity,
                                         bias=bk_sb[:, dkv:dkv + 1])

                # ---- S9: v natural + ones col (vaug) -------------------
                # vaug1 (register rows of v): head h parked at partition
                # base 32*(h%4), slot h//4, so 4 heads' 5-row matmuls can
                # run concurrently in distinct PE row-groups.
                vaug0 = mid.tile([128, H, DH + 1], _bf, tag="vaug0")
                vaug1 = mid.tile([128, 3, DH + 1], _bf, tag="vaug1")
                vtmp = mid.tile([128, D], _bf, tag="vtmp")
                nc.vector.memset(vaug0[:, :, DH:DH + 1], 1.0)
                nc.vector.memset(vaug1[:, :, DH:DH + 1], 1.0)
                for mc, (rows, msl) in enumerate(
                        [(128, slice(0, 128)), (R, slice(128, 133))]):
                    for hf in range(2):
                        psv = pp.tile([128, 512], _f32, tag="ps", name="ps")[:rows, :384]
                        for dc in range(DC):
                            nc.tensor.matmul(
                                psv,
                                lhsT=ctxT_sb[:, dc, msl],
                                rhs=wctx_sb[:, dc, D + 384 * hf:D + 384 * (hf + 1)],
                                start=(dc == 0), stop=(dc == DC - 1))
                        if mc == 0:
                            nc.vector.tensor_add(
                                vaug0[0:rows, 6 * hf:6 * (hf + 1), 0:DH],
                                psv.rearrange("p (h e) -> p h e", e=DH),
                                bvb_sb[0:rows, 384 * hf:384 * (hf + 1)]
                                .rearrange("p (h e) -> p h e", e=DH))
                        else:
                            nc.vector.tensor_add(
                                vtmp[0:rows, 384 * hf:384 * (hf + 1)], psv,
                                bvb_sb[0:rows, 384 * hf:384 * (hf + 1)])
                for h in range(H):
                    nc.sync.dma_start(
                        vaug1[32 * (h % 4):32 * (h % 4) + R, h // 4, 0:DH],
                        vtmp[0:R, DH * h:DH * (h + 1)])

                # ---- S10: qT = (x @ Wq)^T ------------------------------
                # double-buffered so the next sample's q-projection can
                # overlap this sample's MHSA/normalization (keeps PE warm)
                qT_sb = qpool.tile([128, DC, N], _bf, tag="qT")
                for dq in range(DC):
                    for nf in range(3):
                        psq = pp.tile([128, 512], _f32, tag="ps", name="ps")[:, :NF]
                        for dc in range(DC):
                            nc.tensor.matmul(
                                psq,
                                lhsT=wq_sb[:, dc, 128 * dq:128 * (dq + 1)],
                                rhs=xT_sb[:, dc, NF * nf:NF * (nf + 1)],
                                start=(dc == 0), stop=(dc == DC - 1))
                        nc.scalar.activation(
                            qT_sb[:, dq, NF * nf:NF * (nf + 1)], psq,
                            mybir.ActivationFunctionType.Copy)

                # ---- S11: MHSA, head pairs (row-groups 0/64 overlap) ---
                # head h's softmax denominator row lives at partition
                # 32*(h%4), free slot h//4 (32-aligned bases only)
                yTu_sb = qpool.tile([128, DC, N], _bf, tag="yTu")
                S_sb = qpool.tile([128, 3, N], _f32, tag="S")
                nc.gpsimd.memset(S_sb[:], 1.0)
                Ea1s = eap.tile([128, 3, N], _bf, tag="Ea1s")
                for q in range(3):
                    heads = tuple(4 * q + i for i in range(4))
                    Ea0 = {h: attp.tile([128, N], _bf, tag=f"Ea0_{h % 4}",
                                        name="Ea0") for h in heads}
                    for nf in range(3):
                        nsl = slice(NF * nf, NF * (nf + 1))
                        ps0 = {}
                        for h in heads:
                            off = DH * (h % 2)
                            ps0[h] = pa.tile([128, 512], _f32, tag="pa",
                                             name="ps0")[:, :NF]
                            nc.tensor.matmul(
                                ps0[h],
                                lhsT=kT_sb[off:off + DH, h // 2, 0:128],
                                rhs=qT_sb[off:off + DH, h // 2, nsl],
                                start=True, stop=True)
                        for h in heads:
                            nc.scalar.activation(
                                Ea0[h][:, nsl], ps0[h],
                                mybir.ActivationFunctionType.Exp, scale=0.125)
                        # register-token scores of all 4 heads stacked in one
                        # psum bank at bases 0/32/64/96; one exp evicts them
                        # all (rows between bases hold junk, never read)
                        ps1 = pa.tile([128, 512], _f32, tag="pa",
                                      name="ps1")[:, :NF]
                        for h in heads:
                            off = DH * (h % 2)
                            sp = 32 * (h % 4)
                            nc.tensor.matmul(
                                ps1[sp:sp + R, :],
                                lhsT=kT_sb[off:off + DH, h // 2, 128:133],
                                rhs=qT_sb[off:off + DH, h // 2, nsl],
                                start=True, stop=True,
                                tile_position=(off, sp))
                        nc.scalar.activation(
                            Ea1s[0:101, q, nsl], ps1[0:101, :],
                            mybir.ActivationFunctionType.Exp, scale=0.125)
                    for nf in range(3):
                        nsl = slice(NF * nf, NF * (nf + 1))
                        for h in heads:
                            off = DH * (h % 2)
                            sp = 32 * (h % 4)
                            psy = pa.tile([128, 512], _f32, tag="pa",
                                          name="psy")[:DH + 1, :NF]
                            nc.tensor.matmul(psy, lhsT=vaug0[:, h, :],
                                             rhs=Ea0[h][:, nsl],
                                             start=True, stop=False)
                            nc.tensor.matmul(
                                psy, lhsT=vaug1[sp:sp + R, h // 4, :],
                                rhs=Ea1s[sp:sp + R, q, nsl],
                                start=False, stop=True,
                                tile_position=(sp, 0))
                            nc.vector.tensor_copy(
                                yTu_sb[off:off + DH, h // 2, nsl],
                                psy[0:DH, :])
                            nc.vector.tensor_copy(
                                S_sb[sp:sp + 1, h // 4, nsl],
                                psy[DH:DH + 1, :])

                # ---- S12-S15: per N-chunk: RT, normalize via transpose
                # round-trip (in place in yTu), out-projection, DMA out.
                # ncn-major so the fat out-proj matmuls interleave with the
                # transpose/evict ping-pong and keep the PE warm.
                RT_sb = qpool.tile([128, len(NCH), H], _f32, tag="RT")
                for ncn, pn in enumerate(NCH):
                    nsl = slice(128 * ncn, 128 * ncn + pn)
                    ST_n = small.tile([128, H], _f32, tag="STn")
                    for slot in range(3):
                        psf = ppt.tile([128, 128], _f32, tag="pt", name="psf")
                        nc.tensor.transpose(psf[0:pn, 0:97],
                                            S_sb[0:97, slot, nsl],
                                            idf_sb[0:97, 0:97])
                        nc.vector.tensor_copy(
                            ST_n[0:pn, 4 * slot:4 * slot + 4],
                            psf[0:pn, 0:97:32])
                    nc.vector.reciprocal(RT_sb[0:pn, ncn, :], ST_n[0:pn, :])

                    for dc in range(DC):
                        ps1 = ppt.tile([128, 128], _bf, tag="pt", name="ps1")
                        nc.tensor.transpose(ps1[0:pn, :],
                                            yTu_sb[:, dc, nsl], idb_sb[:])
                        ynat = small.tile([128, 128], _bf, tag="ynat")
                        for hh in range(2):
                            nc.scalar.activation(
                                ynat[0:pn, DH * hh:DH * (hh + 1)],
                                ps1[0:pn, DH * hh:DH * (hh + 1)],
                                mybir.ActivationFunctionType.Copy,
                                scale=RT_sb[0:pn, ncn, 2 * dc + hh:2 * dc + hh + 1])
                        ps2 = ppt.tile([128, 128], _bf, tag="pt", name="ps2")
                        nc.tensor.transpose(ps2[:, 0:pn], ynat[0:pn, :],
                                            idb_sb[0:pn, 0:pn])
                        nc.vector.tensor_copy(yTu_sb[:, dc, nsl], ps2[:, 0:pn])

                    o_sb = outp.tile([128, D], _f32, tag="osb")
                    for hf in range(2):
                        pso = pp.tile([128, 512], _f32, tag="ps", name="ps")[:pn, :384]
                        for dc in range(DC):
                            nc.tensor.matmul(
                                pso,
                                lhsT=yTu_sb[:, dc, nsl],
                                rhs=wout_sb[:, dc, 384 * hf:384 * (hf + 1)],
                                start=(dc == 0), stop=(dc == DC - 1))
                        nc.vector.tensor_add(
                            o_sb[0:pn, 384 * hf:384 * (hf + 1)], pso,
                            bob_sb[0:pn, 384 * hf:384 * (hf + 1)])
                    nc.sync.dma_start(out[b, nsl, :], o_sb[0:pn, :])

    nc.compile()
    return nc


def kernel(x, Q_banks, Wq, Wctx, bctx, Wout, bout, centroids):
    global LAST_RESULT
    x = np.asarray(x, np.float32)
    cls = x[:, 0, :]
    nrm = np.linalg.norm(cls, axis=-1, keepdims=True)
    cls_n = cls / np.maximum(nrm, 1e-12)
    sims = cls_n @ np.asarray(centroids, np.float32).T
    idx = np.argmax(sims, axis=-1).astype(np.int32)
    q_ctx = np.asarray(Q_banks, np.float32)[idx]          # [B, K, D]

    xT = np.ascontiguousarray(x.transpose(0, 2, 1)).astype(BF16)
    xpa = np.empty((B, P, D + 1), BF16)
    xpa[:, :, :D] = x[:, R:, :].astype(BF16)
    xpa[:, :, D] = np.asarray(1.0, BF16)
    qcT = np.ascontiguousarray(q_ctx.transpose(0, 2, 1)).astype(BF16)

    bctx = np.asarray(bctx, np.float32)
    bk = np.ascontiguousarray(bctx[:D].reshape(DC, 128).T).astype(np.float32)
    bvb = np.ascontiguousarray(
        np.broadcast_to(bctx[D:], (128, D))).astype(np.float32)
    bob = np.ascontiguousarray(
        np.broadcast_to(np.asarray(bout, np.float32), (128, D))).astype(np.float32)
    shared = {
        "wq": np.asarray(Wq, np.float32).astype(BF16),
        "wctx": np.asarray(Wctx, np.float32).astype(BF16),
        "wout": np.asarray(Wout, np.float32).astype(BF16),
        "bk": bk, "bvb": bvb, "bob": bob,
        "identb": np.eye(128, dtype=np.float32).astype(BF16),
        "identf": np.eye(128, dtype=np.float32),
    }

    nc = build_nc(SPC)
    in_maps = []
    for c in range(NCORES):
        sl = slice(SPC * c, SPC * (c + 1))
        in_maps.append({
            "xT": xT[sl], "xpa": xpa[sl], "qcT": qcT[sl], **shared})

    res = run_bass_kernel_spmd(
        nc, in_maps, list(range(NCORES)), trace=TRACE)
    LAST_RESULT = res
    out = np.concatenate(
        [np.asarray(res.results[c]["out"], np.float32) for c in range(NCORES)],
        axis=0)
    return out, cls_n, idx


# revision 21
# speedup vs baseline: 1.1502x; 1.0781x over previous
"""Trainium2 Bass kernel for nn_ContextAttention (vq_codebook).

Strategy: data-parallel over the batch (B=64) across 8 NeuronCores, 8 samples
per core.  The tiny routing stage (l2-normalize CLS, 4-way argmax, codebook
gather) runs on the host in fp32; everything else (cross-attention pooling,
k/v/q projections, 12-head MHSA, output projection) runs on-device.

Device-side layout choices (per sample):
  - host pre-transposes x -> xT [D, N] so every D-contraction matmul has D on
    partitions without any device transpose of x.
  - cross-attn softmax is computed WITHOUT max subtraction (scores are bounded
    ~|3.5|) in transposed layout scoresT [P, K]; the softmax denominator is
    obtained for free as an extra ones-column in the ctx matmul and folded
    into the PSUM eviction as a per-partition scale.
  - MHSA is computed as attT [m=133, N] per head (m on partitions), exp with
    the 1/8 scale folded into the ACT activation, denominator obtained by
    augmenting V with a ones column, and the per-(head, n) normalization is
    applied during a PE-transpose round-trip (yT -> y_nat(scale) -> yT).
All matmul operands are bf16 (fp32 PSUM accumulation); vector math fp32.
"""
import sys

sys.path.insert(0, "/opt/trn_rl_repo")

import numpy as np
import ml_dtypes

import concourse.bass as bass
import concourse.mybir as mybir
import concourse.tile as tile
from concourse import bacc
from concourse.bass_utils import run_bass_kernel_spmd

BF16 = ml_dtypes.bfloat16

B, N, D = 64, 1029, 768
H, DH = 12, 64
K, R, M = 128, 5, 4
P = N - R            # 1024 patches
NCORES = 8
SPC = B // NCORES    # samples per core
DC = D // 128        # 6 chunks of the model dim
PC = P // 128        # 8 chunks of the patch dim
NF = 343             # N = 3 * 343 free-dim split for big matmuls
NCH = [128] * 8 + [5]  # N partition-chunks: 8*128 + 5

_bf = mybir.dt.bfloat16
_f32 = mybir.dt.float32

# set by test harness: trace the SPMD run and record exec time
TRACE = False
LAST_RESULT = None


def build_nc(samples=SPC, num_cores=NCORES):
    # Bacc (not plain Bass): its compile() runs move_matmul_waits_to_ldweights
    # + generate_event_semaphores, required by TRN2's 1-wait-per-instruction
    # limit (walrus rejects 2-wait matmuls otherwise).
    nc = bacc.Bacc("TRN2", target_bir_lowering=False, debug=False,
                   num_devices=num_cores)

    xT = nc.declare_dram_parameter("xT", [samples, D, N], _bf, isOutput=False)
    xpa = nc.declare_dram_parameter("xpa", [samples, P, D + 1], _bf, isOutput=False)
    qcT = nc.declare_dram_parameter("qcT", [samples, D, K], _bf, isOutput=False)
    wq = nc.declare_dram_parameter("wq", [D, D], _bf, isOutput=False)
    wctx = nc.declare_dram_parameter("wctx", [D, 2 * D], _bf, isOutput=False)
    wout = nc.declare_dram_parameter("wout", [D, D], _bf, isOutput=False)
    bk = nc.declare_dram_parameter("bk", [128, DC], _f32, isOutput=False)
    bvb = nc.declare_dram_parameter("bvb", [128, D], _f32, isOutput=False)
    bob = nc.declare_dram_parameter("bob", [128, D], _f32, isOutput=False)
    identb = nc.declare_dram_parameter("identb", [128, 128], _bf, isOutput=False)
    identf = nc.declare_dram_parameter("identf", [128, 128], _f32, isOutput=False)
    out = nc.declare_dram_parameter("out", [samples, N, D], _f32, isOutput=True)

    with tile.TileContext(nc) as tc:
        with (
            tc.tile_pool(name="wpool", bufs=1) as wpool,
            tc.tile_pool(name="inpool", bufs=2) as inpool,
            tc.tile_pool(name="mid", bufs=1) as mid,
            tc.tile_pool(name="qpool", bufs=2) as qpool,
            tc.tile_pool(name="xpap", bufs=1) as xpap,
            tc.tile_pool(name="small", bufs=2) as small,
            tc.tile_pool(name="attp", bufs=2) as attp,
            tc.tile_pool(name="eap", bufs=1) as eap,
            tc.tile_pool(name="outp", bufs=2) as outp,
            tc.tile_pool(name="pp", bufs=2, space="PSUM") as pp,
            tc.tile_pool(name="pa", bufs=4, space="PSUM") as pa,
            tc.tile_pool(name="ppt", bufs=2, space="PSUM") as ppt,
        ):
            # ---- persistent tiles --------------------------------------
            wq_sb = wpool.tile([128, DC, D], _bf)
            nc.sync.dma_start(wq_sb[:], wq.rearrange("(dc p) q -> p dc q", p=128))
            wctx_sb = wpool.tile([128, DC, 2 * D], _bf)
            nc.sync.dma_start(wctx_sb[:], wctx.rearrange("(dc p) q -> p dc q", p=128))
            wout_sb = wpool.tile([128, DC, D], _bf)
            nc.sync.dma_start(wout_sb[:], wout.rearrange("(dc p) q -> p dc q", p=128))
            bk_sb = wpool.tile([128, DC], _f32)
            nc.sync.dma_start(bk_sb[:], bk[:])
            bvb_sb = wpool.tile([128, D], _f32)
            nc.sync.dma_start(bvb_sb[:], bvb[:])
            bob_sb = wpool.tile([128, D], _f32)
            nc.sync.dma_start(bob_sb[:], bob[:])
            idb_sb = wpool.tile([128, 128], _bf)
            nc.sync.dma_start(idb_sb[:], identb[:])
            idf_sb = wpool.tile([128, 128], _f32)
            nc.sync.dma_start(idf_sb[:], identf[:])

            for b in range(samples):
                # ---- per-sample inputs ---------------------------------
                xT_sb = inpool.tile([128, DC, N], _bf, tag="xT")
                nc.sync.dma_start(
                    xT_sb[:], xT[b].rearrange("(dc p) n -> p dc n", p=128))
                xpa_sb = xpap.tile([128, PC, D + 1], _bf, tag="xpa")
                nc.sync.dma_start(
                    xpa_sb[:], xpa[b].rearrange("(pc p) e -> p pc e", p=128))
                qcT_sb = inpool.tile([128, DC, K], _bf, tag="qcT")
                nc.sync.dma_start(
                    qcT_sb[:], qcT[b].rearrange("(dc p) k -> p dc k", p=128))

                # ---- S2/S3: scoresT + E (cross-attn, transposed) -------
                E_sb = mid.tile([128, PC, K], _bf, tag="E")
                for pc in range(PC):
                    ps_sc = pp.tile([128, 512], _f32, tag="ps", name="ps")[:, :K]
                    for dc in range(DC):
                        nc.tensor.matmul(
                            ps_sc,
                            lhsT=xT_sb[:, dc, R + 128 * pc:R + 128 * (pc + 1)],
                            rhs=qcT_sb[:, dc, :],
                            start=(dc == 0), stop=(dc == DC - 1))
                    nc.scalar.activation(
                        E_sb[:, pc, :], ps_sc, mybir.ActivationFunctionType.Exp)

                # ---- S4-S6: ctx_p = (E^T @ [xp | 1]) * (1/s) -----------
                psA = pp.tile([128, 512], _f32, tag="ps", name="ps")[:, :384]
                psB = pp.tile([128, 512], _f32, tag="ps", name="ps")[:, :385]
                for pc in range(PC):
                    nc.tensor.matmul(psA, lhsT=E_sb[:, pc, :],
                                     rhs=xpa_sb[:, pc, 0:384],
                                     start=(pc == 0), stop=(pc == PC - 1))
                for pc in range(PC):
                    nc.tensor.matmul(psB, lhsT=E_sb[:, pc, :],
                                     rhs=xpa_sb[:, pc, 384:769],
                                     start=(pc == 0), stop=(pc == PC - 1))
                r_sb = small.tile([128, 1], _f32, tag="r")
                nc.vector.reciprocal(r_sb[:], psB[:, 384:385])
                ctxp_sb = mid.tile([128, D], _bf, tag="ctxp")
                nc.scalar.activation(ctxp_sb[:, 0:384], psA,
                                     mybir.ActivationFunctionType.Copy,
                                     scale=r_sb[:])
                nc.scalar.activation(ctxp_sb[:, 384:768], psB[:, 0:384],
                                     mybir.ActivationFunctionType.Copy,
                                     scale=r_sb[:])

                # ---- S7: ctxT = [xregT | ctx_p^T] ----------------------
                ctxT_sb = mid.tile([128, DC, R + K], _bf, tag="ctxT")
                for dc in range(DC):
                    nc.vector.tensor_copy(ctxT_sb[:, dc, 0:R],
                                          xT_sb[:, dc, 0:R])
                    pst = ppt.tile([128, 128], _bf, tag="pt", name="pst")
                    nc.tensor.transpose(
                        pst[:], ctxp_sb[:, 128 * dc:128 * (dc + 1)], idb_sb[:])
                    nc.vector.tensor_copy(ctxT_sb[:, dc, R:R + K], pst[:])

                # ---- S8: kT = (ctx @ Wk)^T + bk ------------------------
                kT_sb = mid.tile([128, DC, R + K], _bf, tag="kT")
                for dkv in range(DC):
                    psk = pp.tile([128, 512], _f32, tag="ps", name="ps")[:, :R + K]
                    for dc in range(DC):
                        nc.tensor.matmul(
                            psk,
                            lhsT=wctx_sb[:, dc, 128 * dkv:128 * (dkv + 1)],
                            rhs=ctxT_sb[:, dc, :],
                            start=(dc == 0), stop=(dc == DC - 1))
                    nc.scalar.activation(kT_sb[:, dkv, :], psk,
                                         mybir.ActivationFunctionType.Identity,
                                         bias=bk_sb[:, dkv:dkv + 1])

                # ---- S9: v natural + ones col (vaug) -------------------
                # vaug1 (register rows of v): head h parked at partition
                # base 32*(h%4), slot h//4, so 4 heads' 5-row matmuls can
                # run concurrently in distinct PE row-groups.
                vaug0 = mid.tile([128, H, DH + 1], _bf, tag="vaug0")
                vaug1 = mid.tile([128, 3, DH + 1], _bf, tag="vaug1")
                vtmp = mid.tile([128, D], _bf, tag="vtmp")
                nc.vector.memset(vaug0[:, :, DH:DH + 1], 1.0)
                nc.vector.memset(vaug1[:, :, DH:DH + 1], 1.0)
                for mc, (rows, msl) in enumerate(
                        [(128, slice(0, 128)), (R, slice(128, 133))]):
                    for hf in range(2):
                        psv = pp.tile([128, 512], _f32, tag="ps", name="ps")[:rows, :384]
                        for dc in range(DC):
                            nc.tensor.matmul(
                                psv,
                                lhsT=ctxT_sb[:, dc, msl],
                                rhs=wctx_sb[:, dc, D + 384 * hf:D + 384 * (hf + 1)],
                                start=(dc == 0), stop=(dc == DC - 1))
                        if mc == 0:
                            nc.vector.tensor_add(
                                vaug0[0:rows, 6 * hf:6 * (hf + 1), 0:DH],
                                psv.rearrange("p (h e) -> p h e", e=DH),
                                bvb_sb[0:rows, 384 * hf:384 * (hf + 1)]
                                .rearrange("p (h e) -> p h e", e=DH))
                        else:
                            nc.vector.tensor_add(
                                vtmp[0:rows, 384 * hf:384 * (hf + 1)], psv,
                                bvb_sb[0:rows, 384 * hf:384 * (hf + 1)])
                for h in range(H):
                    nc.sync.dma_start(
                        vaug1[32 * (h % 4):32 * (h % 4) + R, h // 4, 0:DH],
                        vtmp[0:R, DH * h:DH * (h + 1)])

                # ---- S10: qT = (x @ Wq)^T ------------------------------
                # double-buffered so the next sample's q-projection can
                # overlap this sample's MHSA/normalization (keeps PE warm)
                qT_sb = qpool.tile([128, DC, N], _bf, tag="qT")
                for dq in range(DC):
                    for nf in range(3):
                        psq = pp.tile([128, 512], _f32, tag="ps", name="ps")[:, :NF]
                        for dc in range(DC):
                            nc.tensor.matmul(
                                psq,
                                lhsT=wq_sb[:, dc, 128 * dq:128 * (dq + 1)],
                                rhs=xT_sb[:, dc, NF * nf:NF * (nf + 1)],
                                start=(dc == 0), stop=(dc == DC - 1))
                        nc.scalar.activation(
                            qT_sb[:, dq, NF * nf:NF * (nf + 1)], psq,
                            mybir.ActivationFunctionType.Copy)

                # ---- S11: MHSA, head pairs (row-groups 0/64 overlap) ---
                # head h's softmax denominator row lives at partition
                # 32*(h%4), free slot h//4 (32-aligned bases only)
                yTu_sb = qpool.tile([128, DC, N], _bf, tag="yTu")
                S_sb = qpool.tile([128, 3, N], _f32, tag="S")
                nc.gpsimd.memset(S_sb[:], 1.0)
                Ea1s = eap.tile([128, 3, N], _bf, tag="Ea1s")
                for q in range(3):
                    heads = tuple(4 * q + i for i in range(4))
                    Ea0 = {h: attp.tile([128, N], _bf, tag=f"Ea0_{h % 4}",
                                        name="Ea0") for h in heads}
                    for nf in range(3):
                        nsl = slice(NF * nf, NF * (nf + 1))
                        ps0 = {}
                        for h in heads:
                            off = DH * (h % 2)
                            ps0[h] = pa.tile([128, 512], _f32, tag="pa",
                                             name="ps0")[:, :NF]
                            nc.tensor.matmul(
                                ps0[h],
                                lhsT=kT_sb[off:off + DH, h // 2, 0:128],
                                rhs=qT_sb[off:off + DH, h // 2, nsl],
                                start=True, stop=True)
                        for h in heads:
                            nc.scalar.activation(
                                Ea0[h][:, nsl], ps0[h],
                                mybir.ActivationFunctionType.Exp, scale=0.125)
                        # register-token scores of all 4 heads stacked in one
                        # psum bank at bases 0/32/64/96; one exp evicts them
                        # all (rows between bases hold junk, never read)
                        ps1 = pa.tile([128, 512], _f32, tag="pa",
                                      name="ps1")[:, :NF]
                        for h in heads:
                            off = DH * (h % 2)
                            sp = 32 * (h % 4)
                            nc.tensor.matmul(
                                ps1[sp:sp + R, :],
                                lhsT=kT_sb[off:off + DH, h // 2, 128:133],
                                rhs=qT_sb[off:off + DH, h // 2, nsl],
                                start=True, stop=True,
                                tile_position=(off, sp))
                        nc.scalar.activation(
                            Ea1s[0:101, q, nsl], ps1[0:101, :],
                            mybir.ActivationFunctionType.Exp, scale=0.125)
                    for nf in range(3):
                        nsl = slice(NF * nf, NF * (nf + 1))
                        for h in heads:
                            off = DH * (h % 2)
                            sp = 32 * (h % 4)
                            psy = pa.tile([128, 512], _f32, tag="pa",
                                          name="psy")[:DH + 1, :NF]
                            nc.tensor.matmul(psy, lhsT=vaug0[:, h, :],
                                             rhs=Ea0[h][:, nsl],
                                             start=True, stop=False)
                            nc.tensor.matmul(
                                psy, lhsT=vaug1[sp:sp + R, h // 4, :],
                                rhs=Ea1s[sp:sp + R, q, nsl],
                                start=False, stop=True,
                                tile_position=(sp, 0))
                            nc.vector.tensor_copy(
                                yTu_sb[off:off + DH, h // 2, nsl],
                                psy[0:DH, :])
                            nc.scalar.activation(
                                S_sb[sp:sp + 1, h // 4, nsl],
                                psy[DH:DH + 1, :],
                                mybir.ActivationFunctionType.Copy)

                # ---- S12-S15: per N-chunk: RT, normalize via transpose
                # round-trip (in place in yTu), out-projection, DMA out.
                # ncn-major so the fat out-proj matmuls interleave with the
                # transpose/evict ping-pong and keep the PE warm.
                RT_sb = qpool.tile([128, len(NCH), H], _f32, tag="RT")
                for ncn, pn in enumerate(NCH):
                    nsl = slice(128 * ncn, 128 * ncn + pn)
                    ST_n = small.tile([128, H], _f32, tag="STn")
                    for slot in range(3):
                        psf = ppt.tile([128, 128], _f32, tag="pt", name="psf")
                        nc.tensor.transpose(psf[0:pn, 0:97],
                                            S_sb[0:97, slot, nsl],
                                            idf_sb[0:97, 0:97])
                        nc.vector.tensor_copy(
                            ST_n[0:pn, 4 * slot:4 * slot + 4],
                            psf[0:pn, 0:97:32])
                    nc.vector.reciprocal(RT_sb[0:pn, ncn, :], ST_n[0:pn, :])

                    for dc in range(DC):
                        ps1 = ppt.tile([128, 128], _bf, tag="pt", name="ps1")
                        nc.tensor.transpose(ps1[0:pn, :],
                                            yTu_sb[:, dc, nsl], idb_sb[:])
                        ynat = small.tile([128, 128], _bf, tag="ynat")
                        # one fused scale for both head-halves: in1 is the
                        # [pn, 2] reciprocal pair broadcast along dd (step 0)
                        rt2 = RT_sb[0:pn, ncn, 2 * dc:2 * dc + 2]
                        rt3 = bass.AP(rt2.tensor, rt2.offset,
                                      list(rt2.ap) + [[0, DH]])
                        nc.vector.tensor_tensor(
                            ynat[0:pn, :].rearrange("p (h e) -> p h e", e=DH),
                            ps1[0:pn, :].rearrange("p (h e) -> p h e", e=DH),
                            rt3, mybir.AluOpType.mult)
                        ps2 = ppt.tile([128, 128], _bf, tag="pt", name="ps2")
                        nc.tensor.transpose(ps2[:, 0:pn], ynat[0:pn, :],
                                            idb_sb[0:pn, 0:pn])
                        nc.vector.tensor_copy(yTu_sb[:, dc, nsl], ps2[:, 0:pn])

                    o_sb = outp.tile([128, D], _f32, tag="osb")
                    for hf in range(2):
                        pso = pp.tile([128, 512], _f32, tag="ps", name="ps")[:pn, :384]
                        for dc in range(DC):
                            nc.tensor.matmul(
                                pso,
                                lhsT=yTu_sb[:, dc, nsl],
                                rhs=wout_sb[:, dc, 384 * hf:384 * (hf + 1)],
                                start=(dc == 0), stop=(dc == DC - 1))
                        nc.vector.tensor_add(
                            o_sb[0:pn, 384 * hf:384 * (hf + 1)], pso,
                            bob_sb[0:pn, 384 * hf:384 * (hf + 1)])
                    nc.sync.dma_start(out[b, nsl, :], o_sb[0:pn, :])

    nc.compile()
    return nc


def kernel(x, Q_banks, Wq, Wctx, bctx, Wout, bout, centroids):
    global LAST_RESULT
    x = np.asarray(x, np.float32)
    cls = x[:, 0, :]
    nrm = np.linalg.norm(cls, axis=-1, keepdims=True)
    cls_n = cls / np.maximum(nrm, 1e-12)
    sims = cls_n @ np.asarray(centroids, np.float32).T
    idx = np.argmax(sims, axis=-1).astype(np.int32)
    q_ctx = np.asarray(Q_banks, np.float32)[idx]          # [B, K, D]

    xT = np.ascontiguousarray(x.transpose(0, 2, 1)).astype(BF16)
    xpa = np.empty((B, P, D + 1), BF16)
    xpa[:, :, :D] = x[:, R:, :].astype(BF16)
    xpa[:, :, D] = np.asarray(1.0, BF16)
    qcT = np.ascontiguousarray(q_ctx.transpose(0, 2, 1)).astype(BF16)

    bctx = np.asarray(bctx, np.float32)
    bk = np.ascontiguousarray(bctx[:D].reshape(DC, 128).T).astype(np.float32)
    bvb = np.ascontiguousarray(
        np.broadcast_to(bctx[D:], (128, D))).astype(np.float32)
    bob = np.ascontiguousarray(
        np.broadcast_to(np.asarray(bout, np.float32), (128, D))).astype(np.float32)
    shared = {
        "wq": np.asarray(Wq, np.float32).astype(BF16),
        "wctx": np.asarray(Wctx, np.float32).astype(BF16),
        "wout": np.asarray(Wout, np.float32).astype(BF16),
        "bk": bk, "bvb": bvb, "bob": bob,
        "identb": np.eye(128, dtype=np.float32).astype(BF16),
        "identf": np.eye(128, dtype=np.float32),
    }

    nc = build_nc(SPC)
    in_maps = []
    for c in range(NCORES):
        sl = slice(SPC * c, SPC * (c + 1))
        in_maps.append({
            "xT": xT[sl], "xpa": xpa[sl], "qcT": qcT[sl], **shared})

    res = run_bass_kernel_spmd(
        nc, in_maps, list(range(NCORES)), trace=TRACE)
    LAST_RESULT = res
    out = np.concatenate(
        [np.asarray(res.results[c]["out"], np.float32) for c in range(NCORES)],
        axis=0)
    return out, cls_n, idx
